# revision 34
# baseline (speedup 1.0000x reference)
"""Multi-head attention kernel for 8 Trainium2 NeuronCores (v3, no collective).

Problem: B=2, S=2048, H=8, DK=DV=64, D=512 (nn_MultiHeadAttention).

Sharding: core c owns batch b=c//4 and query rows [512*r, 512*r+512), r=c%4.
Every core computes ALL K'/V' projections locally from the full K/V (which it
must load anyway). The replicated projection work (+17us PE vs the v1 4-way
dedup) eliminates the AllGather that stalled all engines for ~70us in v1.
All matmuls are f16: fp8 was measured at 1.6-3e-2 rel err (threshold 2e-2)
anywhere in the pipeline - relative error of a random-sign dot product does
not shrink with contraction length.

Per-core dataflow (heads processed as 4 pairs of 2):
  QT[p]  = wq[p].T @ qT + bq          [128, 512] f16
  KT[p]  = wk[p].T @ kT + bk          [128, 2048] f16
  V'[t]  = vT[t].T @ wv + bv | 1      [128, 8, 65] f16 (ones col -> denom)
  scoresT= KT[p] tile @ QT[p]         2 heads packed via tile_position
  at     = exp(scoresT/8)             ACT -> f16, no max-subtract (overflow
                                      safe: scores ~ N(0,1))
  o65   += V'[t,h].T @ at[h]          accumulated over t; row 64 = denom
  o2T[p] = o65[0:64] * bcast(1/den)   DVE recip + gpsimd partition_broadcast
  out    = sum_p o2T[p].T @ wo2[p]    K=128 pair-packed matmuls + bo
"""

import numpy as np

B, S, H, DK, DV = 2, 2048, 8, 64, 64
D = H * DV  # 512
NCORES = 8
ROWS = (B * S) // NCORES  # 512 query rows per core
NPAIR = H // 2  # 4 head pairs
NTT = S // 128  # 16 key tiles
NDC = D // 128  # 4 contraction chunks
P = 128
VW = DV + 1  # 65: V columns per head incl. the ones column

_prog = {}


def _build_program():
    from contextlib import ExitStack

    import concourse.mybir as mybir
    import concourse.tile as tile
    from concourse import bacc

    f32 = mybir.dt.float32
    f16 = mybir.dt.float16
    Exp = mybir.ActivationFunctionType.Exp

    nc = bacc.Bacc("TRN2", target_bir_lowering=False, debug=False, num_devices=NCORES)

    qt_d = nc.dram_tensor("qt", [NDC, P, ROWS], f16, kind="ExternalInput").ap()
    kt_d = nc.dram_tensor("kt", [NDC, P, S], f16, kind="ExternalInput").ap()
    vt_d = nc.dram_tensor("vt", [NTT, P, NDC, P], f16, kind="ExternalInput").ap()
    wq_d = nc.dram_tensor("wq", [NDC, P, D], f16, kind="ExternalInput").ap()
    wk_d = nc.dram_tensor("wk", [NDC, P, D], f16, kind="ExternalInput").ap()
    wv_d = nc.dram_tensor("wv", [NDC, P, D], f16, kind="ExternalInput").ap()
    wo_d = nc.dram_tensor("wo", [NPAIR, P, D], f16, kind="ExternalInput").ap()
    bq_d = nc.dram_tensor("bq", [P, NPAIR], f32, kind="ExternalInput").ap()
    bk_d = nc.dram_tensor("bk", [P, NPAIR], f32, kind="ExternalInput").ap()
    bvb_d = nc.dram_tensor("bvb", [1, D], f32, kind="ExternalInput").ap()
    bob_d = nc.dram_tensor("bob", [1, D], f32, kind="ExternalInput").ap()
    out_d = nc.dram_tensor("out", [ROWS // P, P, D], f32, kind="ExternalOutput").ap()

    with tile.TileContext(nc) as tc, ExitStack() as ctx:
        weights = ctx.enter_context(tc.tile_pool(name="weights", bufs=1))
        raw = ctx.enter_context(tc.tile_pool(name="raw", bufs=1))
        acts = ctx.enter_context(tc.tile_pool(name="acts", bufs=1))
        work = ctx.enter_context(tc.tile_pool(name="work", bufs=1))
        # PSUM: sc ring 3x2 banks (scores + all projection/outproj scratch),
        # oa/ob 1 bank each -> exactly 8 banks.
        ps_sc = ctx.enter_context(tc.tile_pool(name="ps_sc", bufs=3, space="PSUM"))
        ps_oa = ctx.enter_context(tc.tile_pool(name="ps_oa", bufs=1, space="PSUM"))
        ps_ob = ctx.enter_context(tc.tile_pool(name="ps_ob", bufs=1, space="PSUM"))

        # ---------------- load phase ----------------
        wq_sb = [weights.tile([P, D], f16, tag=f"wq{c}", name=f"wq{c}") for c in range(NDC)]
        wk_sb = [weights.tile([P, D], f16, tag=f"wk{c}", name=f"wk{c}") for c in range(NDC)]
        wv_sb = [weights.tile([P, D], f16, tag=f"wv{c}", name=f"wv{c}") for c in range(NDC)]
        qt_sb = [raw.tile([P, ROWS], f16, tag=f"qt{c}", name=f"qt{c}") for c in range(NDC)]
        kt_sb = [raw.tile([P, S], f16, tag=f"kt{c}", name=f"kt{c}") for c in range(NDC)]
        vt_sb = [raw.tile([P, NDC, P], f16, tag=f"vt{t}", name=f"vt{t}") for t in range(NTT)]
        wo_sb = [weights.tile([P, D], f16, tag=f"wo{p}", name=f"wo{p}") for p in range(NPAIR)]
        bq_sb = weights.tile([P, NPAIR], f32, tag="bq")
        bk_sb = weights.tile([P, NPAIR], f32, tag="bk")
        bvb_row = weights.tile([1, D], f32, tag="bvb_row")
        bob_row = weights.tile([1, D], f32, tag="bob_row")
        bvb_sb = weights.tile([P, D], f32, tag="bvb")
        bob_sb = weights.tile([P, D], f32, tag="bob")
        # Load order = consumption order; kt is split per key-slab so the
        # first K projection starts after ~1MB instead of the full 2MB.
        # Bias broadcast tiles load as rows and are broadcast on-chip by the
        # otherwise-idle gpsimd.
        # The early-critical tiles are split in half across two DMA queues:
        # a single queue moves only ~25GB/s, so a 128KB tile costs ~5us alone.
        def dma_halved(out, in_):
            n = out.shape[-1]
            nc.sync.dma_start(out=out[..., 0 : n // 2], in_=in_[..., 0 : n // 2])
            nc.sync.dma_start(out=out[..., n // 2 : n], in_=in_[..., n // 2 : n])

        for c in range(NDC):
            dma_halved(wq_sb[c], wq_d[c])
            dma_halved(qt_sb[c], qt_d[c])
        nc.sync.dma_start(out=bq_sb, in_=bq_d)
        for c in range(NDC):
            dma_halved(wk_sb[c], wk_d[c])
        for c in range(NDC):
            dma_halved(kt_sb[c][:, 0:512], kt_d[c, :, 0:512])
        nc.sync.dma_start(out=bk_sb, in_=bk_d)
        for c in range(NDC):
            dma_halved(wv_sb[c], wv_d[c])
        nc.sync.dma_start(out=bvb_row, in_=bvb_d)
        nc.gpsimd.partition_broadcast(bvb_sb, bvb_row, channels=P)
        for t in range(4):  # halve on the chunk dim (contiguous 512B pieces)
            nc.sync.dma_start(out=vt_sb[t][:, 0:2, :], in_=vt_d[t, :, 0:2, :])
            nc.sync.dma_start(out=vt_sb[t][:, 2:4, :], in_=vt_d[t, :, 2:4, :])
        for g in range(1, 4):
            for c in range(NDC):
                nc.sync.dma_start(
                    out=kt_sb[c][:, g * 512 : (g + 1) * 512],
                    in_=kt_d[c, :, g * 512 : (g + 1) * 512],
                )
            for t in range(3 * g + 1, 3 * g + 4):
                nc.sync.dma_start(out=vt_sb[t], in_=vt_d[t])
        for t in range(13, NTT):
            nc.sync.dma_start(out=vt_sb[t], in_=vt_d[t])
        for p in range(NPAIR):
            nc.sync.dma_start(out=wo_sb[p], in_=wo_d[p])
        nc.sync.dma_start(out=bob_row, in_=bob_d)
        nc.gpsimd.partition_broadcast(bob_sb, bob_row, channels=P)

        # ---------------- persistent compute tiles ----------------
        KT = [acts.tile([P, S], f16, tag=f"KT{p}", name=f"KT{p}") for p in range(NPAIR)]
        QT = [acts.tile([P, ROWS], f16, tag=f"QT{p}", name=f"QT{p}") for p in range(NPAIR)]
        o2T = [acts.tile([P, ROWS], f16, tag=f"o2T{p}", name=f"o2T{p}") for p in range(NPAIR)]
        V16 = [acts.tile([P, H, VW], f16, tag=f"V16{t}", name=f"V16{t}") for t in range(NTT)]

        def sc_tile(name):
            return ps_sc.tile([P, 2, ROWS], f32, tag="sc", name=name)

        def proj_q(p):
            ps = sc_tile("ps_q")
            for c in range(NDC):
                nc.tensor.matmul(
                    ps[:, 0, :], lhsT=wq_sb[c][:, p * P : (p + 1) * P], rhs=qt_sb[c],
                    start=(c == 0), stop=(c == NDC - 1),
                )
            nc.vector.tensor_scalar_add(QT[p], ps[:, 0, :], bq_sb[:, p : p + 1])

        def proj_kt(p, g):
            ps = sc_tile("ps_k")
            for c in range(NDC):
                nc.tensor.matmul(
                    ps[:, 0, :],
                    lhsT=wk_sb[c][:, p * P : (p + 1) * P],
                    rhs=kt_sb[c][:, g * 512 : (g + 1) * 512],
                    start=(c == 0), stop=(c == NDC - 1),
                )
            nc.vector.tensor_scalar_add(
                KT[p][:, g * 512 : (g + 1) * 512], ps[:, 0, :], bk_sb[:, p : p + 1]
            )

        def proj_v(t):
            ps = sc_tile("ps_v")
            for c in range(NDC):
                nc.tensor.matmul(
                    ps[:, 0, :], lhsT=vt_sb[t][:, c, :], rhs=wv_sb[c],
                    start=(c == 0), stop=(c == NDC - 1),
                )
            nc.vector.tensor_add(
                V16[t][:, :, 0:DV],
                ps[:, 0, :].rearrange("p (h v) -> p h v", h=H),
                bvb_sb.rearrange("p (h v) -> p h v", h=H),
            )
            nc.vector.memset(V16[t][:, :, DV:VW], 1.0)

        # ---------------- prologue ----------------
        proj_q(0)
        proj_kt(0, 0)
        proj_v(0)
        proj_v(1)

        # ---------------- pair pipeline ----------------
        out_part = []  # held output-projection accumulators (pairs 0-2)
        for p in range(NPAIR):
            oA = ps_oa.tile([VW, ROWS], f32, tag="oa", name="oA")
            oB = ps_ob.tile([VW, ROWS], f32, tag="ob", name="oB")
            for t in range(NTT):
                ts = slice(t * P, (t + 1) * P)
                ps = sc_tile("ps_sc")
                nc.tensor.matmul(
                    ps[:, 0, :], lhsT=KT[p][0:64, ts], rhs=QT[p][0:64, :],
                    start=True, stop=True, tile_position=(0, 0),
                )
                nc.tensor.matmul(
                    ps[:, 1, :], lhsT=KT[p][64:P, ts], rhs=QT[p][64:P, :],
                    start=True, stop=True, tile_position=(64, 0),
                )
                at = work.tile([P, 2, ROWS], f16, tag="at", name="at", bufs=6)
                nc.scalar.activation(at, ps, Exp, scale=1.0 / np.sqrt(DK))
                first, last = (t == 0), (t == NTT - 1)
                nc.tensor.matmul(
                    oA, lhsT=V16[t][:, 2 * p, :], rhs=at[:, 0, :],
                    start=first, stop=last,
                )
                nc.tensor.matmul(
                    oB, lhsT=V16[t][:, 2 * p + 1, :], rhs=at[:, 1, :],
                    start=first, stop=last,
                )

                # Drip-feed remaining projection work AFTER this step's
                # scores/ov so a pending load DMA can't head-of-line-block
                # the in-order PE queue; K slabs are emitted just-in-time
                # (one step before their first consumer).
                if p == 0:
                    if t in (3, 7, 11):
                        proj_kt(0, t // 4 + 1)
                    elif t == 4:
                        proj_q(1)
                    if t < NTT - 2:
                        proj_v(t + 2)
                if p == 1 and t == 0:
                    proj_q(2)
                if p == 2 and t == 0:
                    proj_q(3)
                if p < NPAIR - 1 and 11 <= t < 15:
                    proj_kt(p + 1, t - 11)

            # Normalization: denominator rows (row 64) -> gpsimd broadcast ->
            # fast approximate reciprocal on all lanes -> multiplies into the
            # o2T halves. For pairs 0-2 the o65 accumulators are evicted to
            # SBUF first so the single oa/ob PSUM ring frees in ~1.4us and
            # the chain overlaps the next pair; the last pair (tail-exposed)
            # normalizes directly from PSUM to skip the eviction hop.
            den2 = work.tile([1, 2 * ROWS], f32, tag="den2", name="den2", bufs=2)
            nc.vector.tensor_copy(den2[:, 0:ROWS], oA[DV : DV + 1, :])
            nc.vector.tensor_copy(den2[:, ROWS : 2 * ROWS], oB[DV : DV + 1, :])
            if p < NPAIR - 1:
                o65a = work.tile([VW, ROWS], f32, tag="o65a", name="o65a", bufs=2)
                o65b = work.tile([VW, ROWS], f32, tag="o65b", name="o65b", bufs=2)
                nc.vector.tensor_copy(o65a, oA)
                nc.vector.tensor_copy(o65b, oB)
                srcA, srcB = o65a, o65b
            else:
                # Fill the norm-chain gap: output projection over pairs 0-2
                # runs on the otherwise-idle PE while the chain drains.
                for st in range(ROWS // P):
                    out_part.append(sc_tile(f"ps_out{st}"))
                    for pp in range(NPAIR - 1):
                        nc.tensor.matmul(
                            out_part[st][:, 0, :],
                            lhsT=o2T[pp][:, st * P : (st + 1) * P],
                            rhs=wo_sb[pp],
                            start=(pp == 0), stop=False,
                        )
                srcA, srcB = oA, oB
            denb = work.tile([64, 2, ROWS], f32, tag="denb", name="denb", bufs=2)
            nc.gpsimd.partition_broadcast(denb, den2, channels=64)
            rb = work.tile([64, 2, ROWS], f32, tag="rb", name="rb", bufs=2)
            nc.vector.reciprocal_approx_fast(rb, denb)
            nc.vector.tensor_mul(o2T[p][0:64, :], srcA[0:DV, :], rb[:, 0, :])
            nc.vector.tensor_mul(o2T[p][64:P, :], srcB[0:DV, :], rb[:, 1, :])

        # ---------------- output projection: last-pair contribution --------
        for st in range(ROWS // P):
            nc.tensor.matmul(
                out_part[st][:, 0, :],
                lhsT=o2T[NPAIR - 1][:, st * P : (st + 1) * P],
                rhs=wo_sb[NPAIR - 1],
                start=False, stop=True,
            )
            ot = work.tile([P, D], f32, tag="ot", name="ot", bufs=2)
            nc.vector.tensor_add(ot, out_part[st][:, 0, :], bob_sb)
            nc.sync.dma_start(out=out_d[st], in_=ot)

    nc.compile()
    return nc


def _get_program(repeats=1, hw_loop=0):
    key = (repeats, hw_loop)
    if key not in _prog:
        _prog[key] = _build_program()
    return _prog[key]


def _stage_inputs(queries, keys, values, wq, bq, wk, bk, wv, bv, wo, bo):
    """Host staging: transpose activations to [D, S], chunk contractions,
    per-core query shards. Returns the 8 per-core input dicts."""
    h = np.float16

    qT = [np.ascontiguousarray(queries[b].T) for b in range(B)]
    kT = [np.ascontiguousarray(keys[b].T) for b in range(B)]
    vT = [np.ascontiguousarray(values[b].T) for b in range(B)]

    def chunk(m):  # [512, N] -> [4, 128, N] f16
        return np.ascontiguousarray(m.reshape(NDC, P, m.shape[1])).astype(h)

    wq_m = chunk(np.concatenate([wq[i] for i in range(H)], axis=1))
    wk_m = chunk(np.concatenate([wk[i] for i in range(H)], axis=1))
    wv_m = chunk(np.concatenate([wv[i] for i in range(H)], axis=1))
    wo2 = np.ascontiguousarray(wo.reshape(NPAIR, P, D)).astype(h)
    bq_m = np.ascontiguousarray(bq.reshape(NPAIR, P).T)  # [128, 4]
    bk_m = np.ascontiguousarray(bk.reshape(NPAIR, P).T)
    bvb = np.ascontiguousarray(bv.reshape(1, D)).astype(np.float32)
    bob = np.ascontiguousarray(bo.reshape(1, D)).astype(np.float32)

    kt_b = [chunk(kT[b]) for b in range(B)]
    # vt[t][kappa, c, j] = vT[c*128 + kappa, t*128 + j]
    vt_b = [
        np.ascontiguousarray(
            vT[b].reshape(NDC, P, NTT, P).transpose(2, 1, 0, 3)
        ).astype(h)
        for b in range(B)
    ]

    in_maps = []
    for c in range(NCORES):
        b, r = c // 4, c % 4
        qt_c = chunk(qT[b][:, r * ROWS : (r + 1) * ROWS])
        in_maps.append(
            {
                "qt": qt_c, "kt": kt_b[b], "vt": vt_b[b],
                "wq": wq_m, "wk": wk_m, "wv": wv_m, "wo": wo2,
                "bq": bq_m, "bk": bk_m, "bvb": bvb, "bob": bob,
            }
        )
    return in_maps


def run(trace=False, repeats=1, hw_loop=0, **inputs):
    """Run the kernel; returns (output, BassKernelResults)."""
    from concourse.bass_utils import run_bass_kernel_spmd

    nc = _get_program(repeats, hw_loop)
    in_maps = _stage_inputs(**inputs)
    res = run_bass_kernel_spmd(nc, in_maps, core_ids=list(range(NCORES)), trace=trace)
    out = np.empty((B, S, D), np.float32)
    for c in range(NCORES):
        b, r = c // 4, c % 4
        out[b, r * ROWS : (r + 1) * ROWS, :] = res.results[c]["out"].reshape(ROWS, D)
    return out, res


def kernel(**inputs):
    out, _ = run(trace=False, **inputs)
    return out


# revision 36
# speedup vs baseline: 1.0860x; 1.0860x over previous
"""Multi-head attention kernel for 8 Trainium2 NeuronCores (v3, no collective).

Problem: B=2, S=2048, H=8, DK=DV=64, D=512 (nn_MultiHeadAttention).

Sharding: core c owns batch b=c//4 and query rows [512*r, 512*r+512), r=c%4.
Every core computes ALL K'/V' projections locally from the full K/V (which it
must load anyway). The replicated projection work (+17us PE vs the v1 4-way
dedup) eliminates the AllGather that stalled all engines for ~70us in v1.
All matmuls are f16: fp8 was measured at 1.6-3e-2 rel err (threshold 2e-2)
anywhere in the pipeline - relative error of a random-sign dot product does
not shrink with contraction length.

Per-core dataflow (heads processed as 4 pairs of 2):
  QT[p]  = wq[p].T @ qT + bq          [128, 512] f16
  KT[p]  = wk[p].T @ kT + bk          [128, 2048] f16
  V'[t]  = vT[t].T @ wv + bv | 1      [128, 8, 65] f16 (ones col -> denom)
  scoresT= KT[p] tile @ QT[p]         2 heads packed via tile_position
  at     = exp(scoresT/8)             ACT -> f16, no max-subtract (overflow
                                      safe: scores ~ N(0,1))
  o65   += V'[t,h].T @ at[h]          accumulated over t; row 64 = denom
  o2T[p] = o65[0:64] * bcast(1/den)   DVE recip + gpsimd partition_broadcast
  out    = sum_p o2T[p].T @ wo2[p]    K=128 pair-packed matmuls + bo
"""

import numpy as np

B, S, H, DK, DV = 2, 2048, 8, 64, 64
D = H * DV  # 512
NCORES = 8
ROWS = (B * S) // NCORES  # 512 query rows per core
NPAIR = H // 2  # 4 head pairs
NTT = S // 128  # 16 key tiles
NDC = D // 128  # 4 contraction chunks
P = 128
VW = DV + 1  # 65: V columns per head incl. the ones column

_prog = {}


def _build_program():
    from contextlib import ExitStack

    import concourse.mybir as mybir
    import concourse.tile as tile
    from concourse import bacc

    f32 = mybir.dt.float32
    f16 = mybir.dt.float16
    Exp = mybir.ActivationFunctionType.Exp

    nc = bacc.Bacc("TRN2", target_bir_lowering=False, debug=False, num_devices=NCORES)

    qt_d = nc.dram_tensor("qt", [NDC, P, ROWS], f16, kind="ExternalInput").ap()
    kt_d = nc.dram_tensor("kt", [NDC, P, S], f16, kind="ExternalInput").ap()
    vt_d = nc.dram_tensor("vt", [NTT, P, NDC, P], f16, kind="ExternalInput").ap()
    wq_d = nc.dram_tensor("wq", [NDC, P, D], f16, kind="ExternalInput").ap()
    wk_d = nc.dram_tensor("wk", [NDC, P, D], f16, kind="ExternalInput").ap()
    wv_d = nc.dram_tensor("wv", [NDC, P, D], f16, kind="ExternalInput").ap()
    wo_d = nc.dram_tensor("wo", [NPAIR, P, D], f16, kind="ExternalInput").ap()
    bq_d = nc.dram_tensor("bq", [P, NPAIR], f32, kind="ExternalInput").ap()
    bk_d = nc.dram_tensor("bk", [P, NPAIR], f32, kind="ExternalInput").ap()
    bvb_d = nc.dram_tensor("bvb", [1, D], f32, kind="ExternalInput").ap()
    bob_d = nc.dram_tensor("bob", [1, D], f32, kind="ExternalInput").ap()
    out_d = nc.dram_tensor("out", [ROWS // P, P, D], f32, kind="ExternalOutput").ap()

    with tile.TileContext(nc) as tc, ExitStack() as ctx:
        weights = ctx.enter_context(tc.tile_pool(name="weights", bufs=1))
        raw = ctx.enter_context(tc.tile_pool(name="raw", bufs=1))
        acts = ctx.enter_context(tc.tile_pool(name="acts", bufs=1))
        work = ctx.enter_context(tc.tile_pool(name="work", bufs=1))
        # PSUM: sc ring 3x2 banks (scores + all projection/outproj scratch),
        # oa/ob 1 bank each -> exactly 8 banks.
        ps_sc = ctx.enter_context(tc.tile_pool(name="ps_sc", bufs=3, space="PSUM"))
        ps_oa = ctx.enter_context(tc.tile_pool(name="ps_oa", bufs=1, space="PSUM"))
        ps_ob = ctx.enter_context(tc.tile_pool(name="ps_ob", bufs=1, space="PSUM"))

        # ---------------- load phase ----------------
        wq_sb = [weights.tile([P, D], f16, tag=f"wq{c}", name=f"wq{c}") for c in range(NDC)]
        wk_sb = [weights.tile([P, D], f16, tag=f"wk{c}", name=f"wk{c}") for c in range(NDC)]
        wv_sb = [weights.tile([P, D], f16, tag=f"wv{c}", name=f"wv{c}") for c in range(NDC)]
        qt_sb = [raw.tile([P, ROWS], f16, tag=f"qt{c}", name=f"qt{c}") for c in range(NDC)]
        kt_sb = [raw.tile([P, S], f16, tag=f"kt{c}", name=f"kt{c}") for c in range(NDC)]
        vt_sb = [raw.tile([P, NDC, P], f16, tag=f"vt{t}", name=f"vt{t}") for t in range(NTT)]
        wo_sb = [weights.tile([P, D], f16, tag=f"wo{p}", name=f"wo{p}") for p in range(NPAIR)]
        bq_sb = weights.tile([P, NPAIR], f32, tag="bq")
        bk_sb = weights.tile([P, NPAIR], f32, tag="bk")
        bvb_row = weights.tile([1, D], f32, tag="bvb_row")
        bob_row = weights.tile([1, D], f32, tag="bob_row")
        bvb_sb = weights.tile([P, D], f32, tag="bvb")
        bob_sb = weights.tile([P, D], f32, tag="bob")
        # Load order = consumption order; kt is split per key-slab so the
        # first K projection starts after ~1MB instead of the full 2MB.
        # Bias broadcast tiles load as rows and are broadcast on-chip by the
        # otherwise-idle gpsimd.
        for c in range(NDC):
            nc.sync.dma_start(out=wq_sb[c], in_=wq_d[c])
            nc.sync.dma_start(out=qt_sb[c], in_=qt_d[c])
        nc.sync.dma_start(out=bq_sb, in_=bq_d)
        for c in range(NDC):
            nc.sync.dma_start(out=wk_sb[c], in_=wk_d[c])
        for c in range(NDC):
            nc.sync.dma_start(out=kt_sb[c][:, 0:512], in_=kt_d[c, :, 0:512])
        nc.sync.dma_start(out=bk_sb, in_=bk_d)
        for c in range(NDC):
            nc.sync.dma_start(out=wv_sb[c], in_=wv_d[c])
        nc.sync.dma_start(out=bvb_row, in_=bvb_d)
        nc.gpsimd.partition_broadcast(bvb_sb, bvb_row, channels=P)
        for t in range(4):
            nc.sync.dma_start(out=vt_sb[t], in_=vt_d[t])
        for g in range(1, 4):
            for c in range(NDC):
                nc.sync.dma_start(
                    out=kt_sb[c][:, g * 512 : (g + 1) * 512],
                    in_=kt_d[c, :, g * 512 : (g + 1) * 512],
                )
            for t in range(3 * g + 1, 3 * g + 4):
                nc.sync.dma_start(out=vt_sb[t], in_=vt_d[t])
        for t in range(13, NTT):
            nc.sync.dma_start(out=vt_sb[t], in_=vt_d[t])
        for p in range(NPAIR):
            nc.sync.dma_start(out=wo_sb[p], in_=wo_d[p])
        nc.sync.dma_start(out=bob_row, in_=bob_d)
        nc.gpsimd.partition_broadcast(bob_sb, bob_row, channels=P)

        # ---------------- persistent compute tiles ----------------
        KT = [acts.tile([P, S], f16, tag=f"KT{p}", name=f"KT{p}") for p in range(NPAIR)]
        QT = [acts.tile([P, ROWS], f16, tag=f"QT{p}", name=f"QT{p}") for p in range(NPAIR)]
        o2T = [acts.tile([P, ROWS], f16, tag=f"o2T{p}", name=f"o2T{p}") for p in range(NPAIR)]
        V16 = [acts.tile([P, H, VW], f16, tag=f"V16{t}", name=f"V16{t}") for t in range(NTT)]

        def sc_tile(name):
            return ps_sc.tile([P, 2, ROWS], f32, tag="sc", name=name)

        def proj_q(p):
            ps = sc_tile("ps_q")
            for c in range(NDC):
                nc.tensor.matmul(
                    ps[:, 0, :], lhsT=wq_sb[c][:, p * P : (p + 1) * P], rhs=qt_sb[c],
                    start=(c == 0), stop=(c == NDC - 1),
                )
            nc.vector.tensor_scalar_add(QT[p], ps[:, 0, :], bq_sb[:, p : p + 1])

        def proj_kt(p, g):
            ps = sc_tile("ps_k")
            for c in range(NDC):
                nc.tensor.matmul(
                    ps[:, 0, :],
                    lhsT=wk_sb[c][:, p * P : (p + 1) * P],
                    rhs=kt_sb[c][:, g * 512 : (g + 1) * 512],
                    start=(c == 0), stop=(c == NDC - 1),
                )
            nc.vector.tensor_scalar_add(
                KT[p][:, g * 512 : (g + 1) * 512], ps[:, 0, :], bk_sb[:, p : p + 1]
            )

        def proj_v(t):
            ps = sc_tile("ps_v")
            for c in range(NDC):
                nc.tensor.matmul(
                    ps[:, 0, :], lhsT=vt_sb[t][:, c, :], rhs=wv_sb[c],
                    start=(c == 0), stop=(c == NDC - 1),
                )
            nc.vector.tensor_add(
                V16[t][:, :, 0:DV],
                ps[:, 0, :].rearrange("p (h v) -> p h v", h=H),
                bvb_sb.rearrange("p (h v) -> p h v", h=H),
            )
            nc.vector.memset(V16[t][:, :, DV:VW], 1.0)

        # ---------------- prologue ----------------
        proj_q(0)
        proj_kt(0, 0)
        proj_v(0)
        proj_v(1)

        # ---------------- pair pipeline ----------------
        out_part = []  # held output-projection accumulators (pairs 0-2)
        for p in range(NPAIR):
            oA = ps_oa.tile([VW, ROWS], f32, tag="oa", name="oA")
            oB = ps_ob.tile([VW, ROWS], f32, tag="ob", name="oB")
            for t in range(NTT):
                ts = slice(t * P, (t + 1) * P)
                ps = sc_tile("ps_sc")
                nc.tensor.matmul(
                    ps[:, 0, :], lhsT=KT[p][0:64, ts], rhs=QT[p][0:64, :],
                    start=True, stop=True, tile_position=(0, 0),
                )
                nc.tensor.matmul(
                    ps[:, 1, :], lhsT=KT[p][64:P, ts], rhs=QT[p][64:P, :],
                    start=True, stop=True, tile_position=(64, 0),
                )
                at = work.tile([P, 2, ROWS], f16, tag="at", name="at", bufs=6)
                nc.scalar.activation(at, ps, Exp, scale=1.0 / np.sqrt(DK))
                first, last = (t == 0), (t == NTT - 1)
                nc.tensor.matmul(
                    oA, lhsT=V16[t][:, 2 * p, :], rhs=at[:, 0, :],
                    start=first, stop=last,
                )
                nc.tensor.matmul(
                    oB, lhsT=V16[t][:, 2 * p + 1, :], rhs=at[:, 1, :],
                    start=first, stop=last,
                )

                # Drip-feed remaining projection work AFTER this step's
                # scores/ov so a pending load DMA can't head-of-line-block
                # the in-order PE queue; K slabs are emitted just-in-time
                # (one step before their first consumer).
                if p == 0:
                    if t in (2, 6, 10):
                        proj_kt(0, t // 4 + 1)
                    elif t == 4:
                        proj_q(1)
                    if t < NTT - 2:
                        proj_v(t + 2)
                if p == 1 and t == 0:
                    proj_q(2)
                if p == 2 and t == 0:
                    proj_q(3)
                if p < NPAIR - 1 and 11 <= t < 15:
                    proj_kt(p + 1, t - 11)

            # Normalization: denominator rows (row 64) -> gpsimd broadcast ->
            # fast approximate reciprocal on all lanes -> multiplies into the
            # o2T halves. For pairs 0-2 the o65 accumulators are evicted to
            # SBUF first so the single oa/ob PSUM ring frees in ~1.4us and
            # the chain overlaps the next pair; the last pair (tail-exposed)
            # normalizes directly from PSUM to skip the eviction hop.
            den2 = work.tile([1, 2 * ROWS], f32, tag="den2", name="den2", bufs=2)
            nc.vector.tensor_copy(den2[:, 0:ROWS], oA[DV : DV + 1, :])
            nc.vector.tensor_copy(den2[:, ROWS : 2 * ROWS], oB[DV : DV + 1, :])
            if p < NPAIR - 1:
                o65a = work.tile([VW, ROWS], f32, tag="o65a", name="o65a", bufs=2)
                o65b = work.tile([VW, ROWS], f32, tag="o65b", name="o65b", bufs=2)
                nc.vector.tensor_copy(o65a, oA)
                nc.vector.tensor_copy(o65b, oB)
                srcA, srcB = o65a, o65b
            else:
                # Fill the norm-chain gap: output projection over pairs 0-2
                # runs on the otherwise-idle PE while the chain drains.
                for st in range(ROWS // P):
                    out_part.append(sc_tile(f"ps_out{st}"))
                    for pp in range(NPAIR - 1):
                        nc.tensor.matmul(
                            out_part[st][:, 0, :],
                            lhsT=o2T[pp][:, st * P : (st + 1) * P],
                            rhs=wo_sb[pp],
                            start=(pp == 0), stop=False,
                        )
                srcA, srcB = oA, oB
            denb = work.tile([64, 2, ROWS], f32, tag="denb", name="denb", bufs=2)
            nc.gpsimd.partition_broadcast(denb, den2, channels=64)
            rb = work.tile([64, 2, ROWS], f32, tag="rb", name="rb", bufs=2)
            nc.vector.reciprocal_approx_fast(rb, denb)
            nc.vector.tensor_mul(o2T[p][0:64, :], srcA[0:DV, :], rb[:, 0, :])
            nc.vector.tensor_mul(o2T[p][64:P, :], srcB[0:DV, :], rb[:, 1, :])

        # ---------------- output projection: last-pair contribution --------
        for st in range(ROWS // P):
            nc.tensor.matmul(
                out_part[st][:, 0, :],
                lhsT=o2T[NPAIR - 1][:, st * P : (st + 1) * P],
                rhs=wo_sb[NPAIR - 1],
                start=False, stop=True,
            )
            ot = work.tile([P, D], f32, tag="ot", name="ot", bufs=2)
            nc.vector.tensor_add(ot, out_part[st][:, 0, :], bob_sb)
            nc.sync.dma_start(out=out_d[st], in_=ot)

    nc.compile()
    return nc


def _get_program(repeats=1, hw_loop=0):
    key = (repeats, hw_loop)
    if key not in _prog:
        _prog[key] = _build_program()
    return _prog[key]


def _stage_inputs(queries, keys, values, wq, bq, wk, bk, wv, bv, wo, bo):
    """Host staging: transpose activations to [D, S], chunk contractions,
    per-core query shards. Returns the 8 per-core input dicts."""
    h = np.float16

    qT = [np.ascontiguousarray(queries[b].T) for b in range(B)]
    kT = [np.ascontiguousarray(keys[b].T) for b in range(B)]
    vT = [np.ascontiguousarray(values[b].T) for b in range(B)]

    def chunk(m):  # [512, N] -> [4, 128, N] f16
        return np.ascontiguousarray(m.reshape(NDC, P, m.shape[1])).astype(h)

    wq_m = chunk(np.concatenate([wq[i] for i in range(H)], axis=1))
    wk_m = chunk(np.concatenate([wk[i] for i in range(H)], axis=1))
    wv_m = chunk(np.concatenate([wv[i] for i in range(H)], axis=1))
    wo2 = np.ascontiguousarray(wo.reshape(NPAIR, P, D)).astype(h)
    bq_m = np.ascontiguousarray(bq.reshape(NPAIR, P).T)  # [128, 4]
    bk_m = np.ascontiguousarray(bk.reshape(NPAIR, P).T)
    bvb = np.ascontiguousarray(bv.reshape(1, D)).astype(np.float32)
    bob = np.ascontiguousarray(bo.reshape(1, D)).astype(np.float32)

    kt_b = [chunk(kT[b]) for b in range(B)]
    # vt[t][kappa, c, j] = vT[c*128 + kappa, t*128 + j]
    vt_b = [
        np.ascontiguousarray(
            vT[b].reshape(NDC, P, NTT, P).transpose(2, 1, 0, 3)
        ).astype(h)
        for b in range(B)
    ]

    in_maps = []
    for c in range(NCORES):
        b, r = c // 4, c % 4
        qt_c = chunk(qT[b][:, r * ROWS : (r + 1) * ROWS])
        in_maps.append(
            {
                "qt": qt_c, "kt": kt_b[b], "vt": vt_b[b],
                "wq": wq_m, "wk": wk_m, "wv": wv_m, "wo": wo2,
                "bq": bq_m, "bk": bk_m, "bvb": bvb, "bob": bob,
            }
        )
    return in_maps


def run(trace=False, repeats=1, hw_loop=0, **inputs):
    """Run the kernel; returns (output, BassKernelResults)."""
    from concourse.bass_utils import run_bass_kernel_spmd

    nc = _get_program(repeats, hw_loop)
    in_maps = _stage_inputs(**inputs)
    res = run_bass_kernel_spmd(nc, in_maps, core_ids=list(range(NCORES)), trace=trace)
    out = np.empty((B, S, D), np.float32)
    for c in range(NCORES):
        b, r = c // 4, c % 4
        out[b, r * ROWS : (r + 1) * ROWS, :] = res.results[c]["out"].reshape(ROWS, D)
    return out, res


def kernel(**inputs):
    out, _ = run(trace=False, **inputs)
    return out


# revision 39
# speedup vs baseline: 1.1053x; 1.0178x over previous
"""Multi-head attention kernel for 8 Trainium2 NeuronCores (no collective).

Problem: B=2, S=2048, H=8, DK=DV=64, D=512 (nn_MultiHeadAttention).

Sharding: core c owns batch b=c//4 and query rows [512*r, 512*r+512), r=c%4.
Every core computes ALL K'/V' projections locally from the full K/V (which it
must load anyway). The replicated projection work (+17us PE vs a 4-way
dedup) eliminates the AllGather that stalled all engines for ~70us.
All matmuls are f16: fp8 was measured at 1.6-3e-2 rel err (threshold 2e-2)
anywhere in the pipeline - relative error of a random-sign dot product does
not shrink with contraction length.

Per-core dataflow (heads processed as 4 pairs of 2; projections drip-fed
into the attention pipeline so the load DMAs overlap compute):
  QT[p]  = wq[p].T @ qT + bq          [128, 512] f16
  KT[p]  = wk[p].T @ kT + bk          [128, 2048] f16
  V'[t]  = vT[t].T @ wv + bv | 1      [128, 8, 65] f16 (ones col -> denom)
  scoresT= KT[p] tile @ QT[p]         2 heads packed via tile_position
  at     = exp(scoresT/8)             ACT -> f16, no max-subtract (overflow
                                      safe: scores ~ N(0,1))
  o65   += V'[t,h].T @ at[h]          accumulated over t; row 64 = denom
  o2T[p] = o65[0:64] * bcast(1/den)   gpsimd partition_broadcast of the
                                      denominator rows + DVE fast reciprocal
  out    = sum_p o2T[p].T @ wo2[p]    K=128 pair-packed matmuls + bo;
                                      pairs 0-2 accumulate during the last
                                      pair's normalization window
"""

import numpy as np

B, S, H, DK, DV = 2, 2048, 8, 64, 64
D = H * DV  # 512
NCORES = 8
ROWS = (B * S) // NCORES  # 512 query rows per core
NPAIR = H // 2  # 4 head pairs
NTT = S // 128  # 16 key tiles
NDC = D // 128  # 4 contraction chunks
P = 128
VW = DV + 1  # 65: V columns per head incl. the ones column

_prog = {}


def _build_program():
    from contextlib import ExitStack

    import concourse.mybir as mybir
    import concourse.tile as tile
    from concourse import bacc

    f32 = mybir.dt.float32
    f16 = mybir.dt.float16
    Exp = mybir.ActivationFunctionType.Exp

    nc = bacc.Bacc("TRN2", target_bir_lowering=False, debug=False, num_devices=NCORES)

    qt_d = nc.dram_tensor("qt", [NDC, P, ROWS], f16, kind="ExternalInput").ap()
    kt_d = nc.dram_tensor("kt", [NDC, P, S], f16, kind="ExternalInput").ap()
    vt_d = nc.dram_tensor("vt", [NTT, P, NDC, P], f16, kind="ExternalInput").ap()
    wq_d = nc.dram_tensor("wq", [NDC, P, D], f16, kind="ExternalInput").ap()
    wk_d = nc.dram_tensor("wk", [NDC, P, D], f16, kind="ExternalInput").ap()
    wv_d = nc.dram_tensor("wv", [NDC, P, D], f16, kind="ExternalInput").ap()
    wo_d = nc.dram_tensor("wo", [NPAIR, P, D], f16, kind="ExternalInput").ap()
    bq_d = nc.dram_tensor("bq", [P, NPAIR], f32, kind="ExternalInput").ap()
    bk_d = nc.dram_tensor("bk", [P, NPAIR], f32, kind="ExternalInput").ap()
    bvb_d = nc.dram_tensor("bvb", [1, D], f32, kind="ExternalInput").ap()
    bob_d = nc.dram_tensor("bob", [1, D], f32, kind="ExternalInput").ap()
    out_d = nc.dram_tensor("out", [ROWS // P, P, D], f32, kind="ExternalOutput").ap()

    with tile.TileContext(nc) as tc, ExitStack() as ctx:
        weights = ctx.enter_context(tc.tile_pool(name="weights", bufs=1))
        raw = ctx.enter_context(tc.tile_pool(name="raw", bufs=1))
        acts = ctx.enter_context(tc.tile_pool(name="acts", bufs=1))
        work = ctx.enter_context(tc.tile_pool(name="work", bufs=1))
        # PSUM: sc ring 3x2 banks (scores + all projection/outproj scratch),
        # oa/ob 1 bank each -> exactly 8 banks.
        ps_sc = ctx.enter_context(tc.tile_pool(name="ps_sc", bufs=3, space="PSUM"))
        ps_oa = ctx.enter_context(tc.tile_pool(name="ps_oa", bufs=1, space="PSUM"))
        ps_ob = ctx.enter_context(tc.tile_pool(name="ps_ob", bufs=1, space="PSUM"))

        # ---------------- load phase ----------------
        wq_sb = [weights.tile([P, D], f16, tag=f"wq{c}", name=f"wq{c}") for c in range(NDC)]
        wk_sb = [weights.tile([P, D], f16, tag=f"wk{c}", name=f"wk{c}") for c in range(NDC)]
        wv_sb = [weights.tile([P, D], f16, tag=f"wv{c}", name=f"wv{c}") for c in range(NDC)]
        qt_sb = [raw.tile([P, ROWS], f16, tag=f"qt{c}", name=f"qt{c}") for c in range(NDC)]
        kt_sb = [raw.tile([P, S], f16, tag=f"kt{c}", name=f"kt{c}") for c in range(NDC)]
        vt_sb = [raw.tile([P, NDC, P], f16, tag=f"vt{t}", name=f"vt{t}") for t in range(NTT)]
        wo_sb = [weights.tile([P, D], f16, tag=f"wo{p}", name=f"wo{p}") for p in range(NPAIR)]
        bq_sb = weights.tile([P, NPAIR], f32, tag="bq")
        bk_sb = weights.tile([P, NPAIR], f32, tag="bk")
        bvb_row = weights.tile([1, D], f32, tag="bvb_row")
        bob_row = weights.tile([1, D], f32, tag="bob_row")
        bvb_sb = weights.tile([P, D], f32, tag="bvb")
        bob_sb = weights.tile([P, D], f32, tag="bob")
        # Load order = consumption order; kt is split per key-slab so the
        # first K projection starts after ~1MB instead of the full 2MB.
        # Bias broadcast tiles load as rows and are broadcast on-chip by the
        # otherwise-idle gpsimd.
        for c in range(NDC):
            nc.sync.dma_start(out=wq_sb[c], in_=wq_d[c])
            nc.sync.dma_start(out=qt_sb[c], in_=qt_d[c])
        nc.sync.dma_start(out=bq_sb, in_=bq_d)
        for c in range(NDC):
            nc.sync.dma_start(out=wk_sb[c], in_=wk_d[c])
        for c in range(NDC):
            nc.sync.dma_start(out=kt_sb[c][:, 0:512], in_=kt_d[c, :, 0:512])
        nc.sync.dma_start(out=bk_sb, in_=bk_d)
        for c in range(NDC):
            nc.sync.dma_start(out=wv_sb[c], in_=wv_d[c])
        nc.sync.dma_start(out=bvb_row, in_=bvb_d)
        nc.gpsimd.partition_broadcast(bvb_sb, bvb_row, channels=P)
        for t in range(4):
            nc.sync.dma_start(out=vt_sb[t], in_=vt_d[t])
        for g in range(1, 4):
            for c in range(NDC):
                nc.sync.dma_start(
                    out=kt_sb[c][:, g * 512 : (g + 1) * 512],
                    in_=kt_d[c, :, g * 512 : (g + 1) * 512],
                )
            for t in range(3 * g + 1, 3 * g + 4):
                nc.sync.dma_start(out=vt_sb[t], in_=vt_d[t])
        for t in range(13, NTT):
            nc.sync.dma_start(out=vt_sb[t], in_=vt_d[t])
        for p in range(NPAIR):
            nc.sync.dma_start(out=wo_sb[p], in_=wo_d[p])
        nc.sync.dma_start(out=bob_row, in_=bob_d)
        nc.gpsimd.partition_broadcast(bob_sb, bob_row, channels=P)

        # ---------------- persistent compute tiles ----------------
        KT = [acts.tile([P, S], f16, tag=f"KT{p}", name=f"KT{p}") for p in range(NPAIR)]
        QT = [acts.tile([P, ROWS], f16, tag=f"QT{p}", name=f"QT{p}") for p in range(NPAIR)]
        o2T = [acts.tile([P, ROWS], f16, tag=f"o2T{p}", name=f"o2T{p}") for p in range(NPAIR)]
        V16 = [acts.tile([P, H, VW], f16, tag=f"V16{t}", name=f"V16{t}") for t in range(NTT)]

        def sc_tile(name):
            return ps_sc.tile([P, 2, ROWS], f32, tag="sc", name=name)

        def proj_q(p):
            ps = sc_tile("ps_q")
            for c in range(NDC):
                nc.tensor.matmul(
                    ps[:, 0, :], lhsT=wq_sb[c][:, p * P : (p + 1) * P], rhs=qt_sb[c],
                    start=(c == 0), stop=(c == NDC - 1),
                )
            nc.vector.tensor_scalar_add(QT[p], ps[:, 0, :], bq_sb[:, p : p + 1])

        def proj_kt(p, g):
            ps = sc_tile("ps_k")
            for c in range(NDC):
                nc.tensor.matmul(
                    ps[:, 0, :],
                    lhsT=wk_sb[c][:, p * P : (p + 1) * P],
                    rhs=kt_sb[c][:, g * 512 : (g + 1) * 512],
                    start=(c == 0), stop=(c == NDC - 1),
                )
            nc.vector.tensor_scalar_add(
                KT[p][:, g * 512 : (g + 1) * 512], ps[:, 0, :], bk_sb[:, p : p + 1]
            )

        def proj_v(t):
            ps = sc_tile("ps_v")
            for c in range(NDC):
                nc.tensor.matmul(
                    ps[:, 0, :], lhsT=vt_sb[t][:, c, :], rhs=wv_sb[c],
                    start=(c == 0), stop=(c == NDC - 1),
                )
            nc.vector.tensor_add(
                V16[t][:, :, 0:DV],
                ps[:, 0, :].rearrange("p (h v) -> p h v", h=H),
                bvb_sb.rearrange("p (h v) -> p h v", h=H),
            )
            nc.vector.memset(V16[t][:, :, DV:VW], 1.0)

        # ---------------- prologue ----------------
        proj_q(0)
        proj_kt(0, 0)
        proj_v(0)
        proj_v(1)

        # ---------------- pair pipeline ----------------
        out_part = []  # held output-projection accumulators (pairs 0-2)
        for p in range(NPAIR):
            oA = ps_oa.tile([VW, ROWS], f32, tag="oa", name="oA")
            oB = ps_ob.tile([VW, ROWS], f32, tag="ob", name="oB")
            for t in range(NTT):
                # drip-feed remaining projection work into the pair windows
                # (K slabs staged so their DMAs have time to land)
                if p == 0:
                    if t in (2, 6, 10):
                        proj_kt(0, t // 4 + 1)
                    elif t == 4:
                        proj_q(1)
                    if t < NTT - 2:
                        proj_v(t + 2)
                if p == 1 and t == 0:
                    proj_q(2)
                if p == 2 and t == 0:
                    proj_q(3)
                if p < NPAIR - 1 and 11 <= t < 15:
                    proj_kt(p + 1, t - 11)

                ts = slice(t * P, (t + 1) * P)
                ps = sc_tile("ps_sc")
                nc.tensor.matmul(
                    ps[:, 0, :], lhsT=KT[p][0:64, ts], rhs=QT[p][0:64, :],
                    start=True, stop=True, tile_position=(0, 0),
                )
                nc.tensor.matmul(
                    ps[:, 1, :], lhsT=KT[p][64:P, ts], rhs=QT[p][64:P, :],
                    start=True, stop=True, tile_position=(64, 0),
                )
                at = work.tile([P, 2, ROWS], f16, tag="at", name="at", bufs=6)
                nc.scalar.activation(at, ps, Exp, scale=1.0 / np.sqrt(DK))
                first, last = (t == 0), (t == NTT - 1)
                nc.tensor.matmul(
                    oA, lhsT=V16[t][:, 2 * p, :], rhs=at[:, 0, :],
                    start=first, stop=last,
                )
                nc.tensor.matmul(
                    oB, lhsT=V16[t][:, 2 * p + 1, :], rhs=at[:, 1, :],
                    start=first, stop=last,
                )

            # Normalization: denominator rows (row 64) -> gpsimd broadcast ->
            # fast approximate reciprocal on all lanes -> multiplies into the
            # o2T halves. For pairs 0-2 the o65 accumulators are evicted to
            # SBUF first so the single oa/ob PSUM ring frees in ~1.4us and
            # the chain overlaps the next pair; the last pair (tail-exposed)
            # normalizes directly from PSUM to skip the eviction hop.
            den2 = work.tile([1, 2 * ROWS], f32, tag="den2", name="den2", bufs=2)
            nc.vector.tensor_copy(den2[:, 0:ROWS], oA[DV : DV + 1, :])
            nc.vector.tensor_copy(den2[:, ROWS : 2 * ROWS], oB[DV : DV + 1, :])
            if p < NPAIR - 1:
                o65a = work.tile([VW, ROWS], f32, tag="o65a", name="o65a", bufs=2)
                o65b = work.tile([VW, ROWS], f32, tag="o65b", name="o65b", bufs=2)
                nc.vector.tensor_copy(o65a, oA)
                nc.vector.tensor_copy(o65b, oB)
                srcA, srcB = o65a, o65b
            else:
                # Fill the norm-chain gap: output projection over pairs 0-2
                # runs on the otherwise-idle PE while the chain drains.
                for st in range(ROWS // P):
                    out_part.append(sc_tile(f"ps_out{st}"))
                    for pp in range(NPAIR - 1):
                        nc.tensor.matmul(
                            out_part[st][:, 0, :],
                            lhsT=o2T[pp][:, st * P : (st + 1) * P],
                            rhs=wo_sb[pp],
                            start=(pp == 0), stop=False,
                        )
                srcA, srcB = oA, oB
            denb = work.tile([64, 2, ROWS], f32, tag="denb", name="denb", bufs=2)
            nc.gpsimd.partition_broadcast(denb, den2, channels=64)
            rb = work.tile([64, 2, ROWS], f32, tag="rb", name="rb", bufs=2)
            nc.vector.reciprocal_approx_fast(rb, denb)
            nc.vector.tensor_mul(o2T[p][0:64, :], srcA[0:DV, :], rb[:, 0, :])
            nc.vector.tensor_mul(o2T[p][64:P, :], srcB[0:DV, :], rb[:, 1, :])

        # ---------------- output projection: last-pair contribution --------
        for st in range(ROWS // P):
            nc.tensor.matmul(
                out_part[st][:, 0, :],
                lhsT=o2T[NPAIR - 1][:, st * P : (st + 1) * P],
                rhs=wo_sb[NPAIR - 1],
                start=False, stop=True,
            )
            ot = work.tile([P, D], f32, tag="ot", name="ot", bufs=2)
            nc.vector.tensor_add(ot, out_part[st][:, 0, :], bob_sb)
            nc.sync.dma_start(out=out_d[st], in_=ot)

    nc.compile()
    return nc


def _get_program(repeats=1, hw_loop=0):
    key = (repeats, hw_loop)
    if key not in _prog:
        _prog[key] = _build_program()
    return _prog[key]


def _stage_inputs(queries, keys, values, wq, bq, wk, bk, wv, bv, wo, bo):
    """Host staging: transpose activations to [D, S], chunk contractions,
    per-core query shards. Returns the 8 per-core input dicts."""
    h = np.float16

    qT = [np.ascontiguousarray(queries[b].T) for b in range(B)]
    kT = [np.ascontiguousarray(keys[b].T) for b in range(B)]
    vT = [np.ascontiguousarray(values[b].T) for b in range(B)]

    def chunk(m):  # [512, N] -> [4, 128, N] f16
        return np.ascontiguousarray(m.reshape(NDC, P, m.shape[1])).astype(h)

    wq_m = chunk(np.concatenate([wq[i] for i in range(H)], axis=1))
    wk_m = chunk(np.concatenate([wk[i] for i in range(H)], axis=1))
    wv_m = chunk(np.concatenate([wv[i] for i in range(H)], axis=1))
    wo2 = np.ascontiguousarray(wo.reshape(NPAIR, P, D)).astype(h)
    bq_m = np.ascontiguousarray(bq.reshape(NPAIR, P).T)  # [128, 4]
    bk_m = np.ascontiguousarray(bk.reshape(NPAIR, P).T)
    bvb = np.ascontiguousarray(bv.reshape(1, D)).astype(np.float32)
    bob = np.ascontiguousarray(bo.reshape(1, D)).astype(np.float32)

    kt_b = [chunk(kT[b]) for b in range(B)]
    # vt[t][kappa, c, j] = vT[c*128 + kappa, t*128 + j]
    vt_b = [
        np.ascontiguousarray(
            vT[b].reshape(NDC, P, NTT, P).transpose(2, 1, 0, 3)
        ).astype(h)
        for b in range(B)
    ]

    in_maps = []
    for c in range(NCORES):
        b, r = c // 4, c % 4
        qt_c = chunk(qT[b][:, r * ROWS : (r + 1) * ROWS])
        in_maps.append(
            {
                "qt": qt_c, "kt": kt_b[b], "vt": vt_b[b],
                "wq": wq_m, "wk": wk_m, "wv": wv_m, "wo": wo2,
                "bq": bq_m, "bk": bk_m, "bvb": bvb, "bob": bob,
            }
        )
    return in_maps


def run(trace=False, repeats=1, hw_loop=0, **inputs):
    """Run the kernel; returns (output, BassKernelResults)."""
    from concourse.bass_utils import run_bass_kernel_spmd

    nc = _get_program(repeats, hw_loop)
    in_maps = _stage_inputs(**inputs)
    res = run_bass_kernel_spmd(nc, in_maps, core_ids=list(range(NCORES)), trace=trace)
    out = np.empty((B, S, D), np.float32)
    for c in range(NCORES):
        b, r = c // 4, c % 4
        out[b, r * ROWS : (r + 1) * ROWS, :] = res.results[c]["out"].reshape(ROWS, D)
    return out, res


def kernel(**inputs):
    out, _ = run(trace=False, **inputs)
    return out
